# revision 11
# baseline (speedup 1.0000x reference)
"""DMI-CE loss kernel for Trainium2 (8 NeuronCores, data-parallel over batch).

Problem: pred [256, 4, 16384] f32 logits, labels [256, 16384] i32 in {0,1,2,3}
(3 = pad/ignore).  Loss = 0.1 * mean_b(dmi_b) + CE where
  CE    = -(sum_valid logsoftmax(pred)[y]) / n_valid
  dmi_b = -sign(det(mat_b)) * log(|det(mat_b)| + 1e-3)
  mat_b = onehot(y)^T @ softmax(pred[:, :3]) / j_b   (over the valid prefix)

Sharding: pure data parallel, 32 samples per core.  Inputs are host-packed to
bf16 (labels {0,1,2,3} are exact in bf16), halving HBM traffic; every
device-side reduction lands in a per-(sample,quarter) f32 accumulator column
and the tiny [128, w] partials are combined on host (3x3 dets in f64).

Layout on core: partition p = b_local*4 + hi (hi = token-axis quarter), free
dim = token-in-chunk.  Per chunk the work is split across three engines:
  ACT:   e = exp(pred) (one fused [128, 4F] pass), l4 = ln(s4)
  DVE:   everything else as bf16 scalar_tensor_tensor ops (4x_2p DVE
         mode): class sums s3/s4, q_d = e_d/s3 (ALU divide), and fused
         mask-product reductions (y==c)*q_d, (y==c)*pred_c, (y==c),
         (y<3)*ln(s4) via the instruction's accum_out (f32 scalars).
No ACT accum_out is used (saves the 187ns accumulator-read per op); the
elementwise outputs of reduction ops go to one DVE-only scratch plane
(same-engine WAW is free on the in-order queue).  ln / vl lag one chunk so
neither in-order engine ever stalls on the other inside a chunk.
"""

import numpy as np

import concourse.bass as bass
import concourse.bacc as bacc
import concourse.tile as tile
from concourse import mybir
from concourse.bass_utils import run_bass_kernel_spmd

N_CORES = 8
B, C, L = 256, 4, 16384
B_LOC = B // N_CORES  # 32 samples per core
HI = 4                # token-axis quarters per sample; partition p = b*HI + hi
M = L // HI           # 4096 tokens per partition row
NCHUNK = 4
FC = M // NCHUNK      # 1024 tokens per chunk

# accumulator columns per chunk (all DVE-written):
#   0..2: n_c    3..8: m_cd (3 + 2c + d)    9..11: pk_c    12: vl
AW = 13

IGNORE = 3
DMICE_P = 0.1

# test.py toggles TRACE to get exec_time_ns out of the NTFF profile.
TRACE = False
LAST_EXEC_NS = None
LAST_TRACE_PATH = None

_CACHE = {}

ACT_SET = "natural_log_exp_and_others"  # holds Exp and Ln together


class _Bacc(bacc.Bacc):
    """Bacc whose act-table pass sees only one (correctly-indexed) set.

    The stock pass resolves each activation to the first table set
    containing its function, which ping-pongs Exp<->Ln between different
    sets and inserts a ~1.3us ACT_TABLE_LOAD per transition.  Both
    functions this kernel uses live together in natural_log_exp_and_others,
    so present every other set as empty; ids stay positional, so the
    emitted act_func_set_id is unchanged.
    """

    def insert_act_table_loads(self):
        from concourse import mybir as _mb
        from concourse.hw_specs import get_activation_tables
        import bass_rust as _bass_rust
        has_activation = any(
            isinstance(i, _mb.InstActivation)
            for b in self.main_func.blocks
            for i in b.instructions
        )
        if not has_activation:
            return
        tables = [
            (name, funcs if name == ACT_SET else set())
            for name, funcs in get_activation_tables(self.m.arch).items()
        ]
        _bass_rust.insert_act_table_loads(self, tables)


def _build():
    f32 = mybir.dt.float32
    bf16 = mybir.dt.bfloat16
    i16 = mybir.dt.int16
    Alu = mybir.AluOpType
    Act = mybir.ActivationFunctionType

    nc = _Bacc("TRN2", debug=False, target_bir_lowering=False,
               num_devices=N_CORES)
    pred_d = nc.dram_tensor("pred", [B_LOC, C, L], bf16, kind="ExternalInput")
    lab_d = nc.dram_tensor("labels", [B_LOC, L], bf16, kind="ExternalInput")
    acc_d = nc.dram_tensor("acc", [128, NCHUNK * AW], f32,
                           kind="ExternalOutput")

    # 4-D DRAM APs iterated b -> hi -> (c) -> m; SBUF side is [128=(b,hi), ...]
    # in the same order, so a plain dma_start matches element-for-element.
    pred_v = pred_d.ap().rearrange("b c (h m) -> b h c m", h=HI)
    lab_v = lab_d.ap().rearrange("b (h m) -> b h m", h=HI)

    with tile.TileContext(nc) as tc:
        with (
            tc.tile_pool(name="io", bufs=3) as io_pool,
            tc.tile_pool(name="ep", bufs=2) as e_pool,
            tc.tile_pool(name="mid", bufs=3) as mid_pool,
            tc.tile_pool(name="scr", bufs=1) as scr_pool,
            tc.tile_pool(name="accp", bufs=1) as acc_pool,
        ):
            acc = acc_pool.tile([128, NCHUNK * AW], f32)
            # scratch plane for ops whose elementwise output is unused;
            # DVE-only, and same-engine WAW is free (in-order queue).
            scrD = scr_pool.tile([128, FC], bf16)
            ones = scr_pool.tile([128, FC], bf16)
            twos = scr_pool.tile([128, FC], bf16)
            nc.vector.memset(ones[:], 1.0)
            nc.vector.memset(twos[:], 2.0)

            # Software-pipelined emission: per-engine program order matters
            # (in-order queues).  ACT: exp_k then ln_{k-1}.  DVE: counts and
            # picked logits first (DMA-dep only), then class sums / softmax3
            # divides (exp-dep), mask-products, and the lagged vl_{k-1}
            # (needs the lagged ACT ln).
            yts, l4s, s4s = {}, {}, {}

            def emit_ln(j):
                nc.scalar.activation(l4s[j][:], s4s[j][:], Act.Ln)

            def emit_vl(j):
                nc.vector.scalar_tensor_tensor(
                    scrD[:], yts[j][:], float(IGNORE), l4s[j][:],
                    Alu.is_lt, Alu.mult,
                    accum_out=acc[:, j * AW + 12:j * AW + 13])

            for k in range(NCHUNK):
                lo = k * FC
                yt = io_pool.tile([128, FC], bf16, tag="yt")
                pt = io_pool.tile([128, C, FC], bf16, tag="pt")
                yts[k] = yt
                nc.sync.dma_start(out=yt[:], in_=lab_v[:, :, lo:lo + FC])
                for c in range(C):
                    nc.sync.dma_start(out=pt[:, c, :],
                                      in_=pred_v[:, :, c, lo:lo + FC])

                # --- ACT: all four exps in one pass; lagged ln
                et = e_pool.tile([128, C, FC], bf16, tag="et")
                nc.scalar.activation(et[:, :, :], pt[:, :, :], Act.Exp)
                if k >= 1:
                    emit_ln(k - 1)

                # --- DVE: DMA-dependent accums first (overlap with exp)
                for c in range(3):
                    nc.vector.scalar_tensor_tensor(
                        scrD[:], yt[:], float(c), ones[:], Alu.is_equal,
                        Alu.mult,
                        accum_out=acc[:, k * AW + c:k * AW + c + 1])
                for c in range(3):
                    nc.vector.scalar_tensor_tensor(
                        scrD[:], yt[:], float(c), pt[:, c, :],
                        Alu.is_equal, Alu.mult,
                        accum_out=acc[:, k * AW + 9 + c:k * AW + 10 + c])

                # --- DVE: class sums + softmax3 + mask-product accums
                s01 = mid_pool.tile([128, FC], bf16, tag="s01")
                s3 = mid_pool.tile([128, FC], bf16, tag="s3")
                s4 = mid_pool.tile([128, FC], bf16, tag="s4")
                s4s[k] = s4
                nc.vector.scalar_tensor_tensor(
                    s01[:], et[:, 0, :], 0.0, et[:, 1, :], Alu.add, Alu.add)
                nc.vector.scalar_tensor_tensor(
                    s3[:], s01[:], 0.0, et[:, 2, :], Alu.add, Alu.add)
                nc.vector.scalar_tensor_tensor(
                    s4[:], s3[:], 0.0, et[:, 3, :], Alu.add, Alu.add)

                # Newton-Raphson reciprocal of s3 (no divide in the DVE ISA).
                # Seed via the exponent-flip bit trick, computed on ACT as
                # float math on the int16 view (values < 2^24, exact):
                #   r0_bits = 0x7EF3 - s3_bits
                r0 = mid_pool.tile([128, FC], bf16, tag="r0")
                nc.scalar.activation(r0[:].bitcast(i16), s3[:].bitcast(i16),
                                     Act.Copy, scale=-1.0, bias=float(0x7EF3))
                t = mid_pool.tile([128, FC], bf16, tag="t")
                u = mid_pool.tile([128, FC], bf16, tag="u")
                r1 = mid_pool.tile([128, FC], bf16, tag="r1")
                nc.vector.scalar_tensor_tensor(
                    t[:], s3[:], 0.0, r0[:], Alu.add, Alu.mult)
                nc.vector.scalar_tensor_tensor(
                    u[:], t[:], -1.0, twos[:], Alu.mult, Alu.add)
                nc.vector.scalar_tensor_tensor(
                    r1[:], r0[:], 0.0, u[:], Alu.add, Alu.mult)
                q0 = mid_pool.tile([128, FC], bf16, tag="q0")
                q1 = mid_pool.tile([128, FC], bf16, tag="q1")
                nc.vector.scalar_tensor_tensor(
                    q0[:], et[:, 0, :], 0.0, r1[:], Alu.add, Alu.mult)
                nc.vector.scalar_tensor_tensor(
                    q1[:], et[:, 1, :], 0.0, r1[:], Alu.add, Alu.mult)
                l4s[k] = mid_pool.tile([128, FC], bf16, tag="l4", name="l4")
                qs = (q0, q1)
                for c in range(3):
                    for d in range(2):
                        col = k * AW + 3 + 2 * c + d
                        nc.vector.scalar_tensor_tensor(
                            scrD[:], yt[:], float(c), qs[d][:],
                            Alu.is_equal, Alu.mult,
                            accum_out=acc[:, col:col + 1])
                if k >= 1:
                    emit_vl(k - 1)

            emit_ln(NCHUNK - 1)
            emit_vl(NCHUNK - 1)

            nc.sync.dma_start(out=acc_d.ap(), in_=acc[:])
    nc.compile()
    return nc


def _get_nc():
    if "nc" not in _CACHE:
        _CACHE["nc"] = _build()
    return _CACHE["nc"]


def _finalize(acc_list):
    """Per-core [128, NCHUNK*13] f32 partials -> scalar loss (f64 host math)."""
    per = []
    for a in acc_list:
        a = a.astype(np.float64).reshape(B_LOC, HI, NCHUNK, AW).sum(axis=(1, 2))
        per.append(a)
    a = np.concatenate(per, axis=0)     # [256, 13]
    n_c = a[:, 0:3]                     # per-class valid-token counts
    m01 = a[:, 3:9].reshape(B, 3, 2)    # mat[:, c, 0:2] unnormalized
    pk_total = a[:, 9:12].sum()
    vl_total = a[:, 12].sum()
    mat_u = np.concatenate(
        [m01, (n_c - m01.sum(axis=2))[:, :, None]], axis=2)  # [B, 3, 3]
    j = n_c.sum(axis=1)
    mat = mat_u / j[:, None, None]
    det = np.linalg.det(mat)
    dmi = np.where(det < 0, np.log(np.abs(det) + 1e-3),
                   -np.log(np.abs(det) + 1e-3))
    ce = (vl_total - pk_total) / j.sum()
    loss = DMICE_P * (dmi.sum() / B) + ce
    return np.asarray(loss, dtype=np.float32)


def kernel(pred, labels):
    global LAST_EXEC_NS, LAST_TRACE_PATH
    import ml_dtypes
    bf16 = ml_dtypes.bfloat16
    pred = np.asarray(pred, dtype=np.float32).astype(bf16)
    labels = np.asarray(labels, dtype=np.int32).astype(bf16)
    assert pred.shape == (B, C, L) and labels.shape == (B, L)
    nc = _get_nc()
    in_maps = [
        {
            "pred": np.ascontiguousarray(pred[i * B_LOC:(i + 1) * B_LOC]),
            "labels": np.ascontiguousarray(labels[i * B_LOC:(i + 1) * B_LOC]),
        }
        for i in range(N_CORES)
    ]
    res = run_bass_kernel_spmd(nc, in_maps, core_ids=list(range(N_CORES)),
                               trace=TRACE)
    LAST_EXEC_NS = res.exec_time_ns
    if res.instructions_and_trace is not None:
        LAST_TRACE_PATH = res.instructions_and_trace[1]
    return _finalize([r["acc"] for r in res.results])


if __name__ == "__main__":
    nc = _build()
    print("build ok")


# revision 12
# speedup vs baseline: 1.9218x; 1.9218x over previous
"""DMI-CE loss kernel for Trainium2 (8 NeuronCores, data-parallel over batch).

Problem: pred [256, 4, 16384] f32 logits, labels [256, 16384] i32 in {0,1,2,3}
(3 = pad/ignore).  Loss = 0.1 * mean_b(dmi_b) + CE where
  CE    = -(sum_valid logsoftmax(pred)[y]) / n_valid
  dmi_b = -sign(det(mat_b)) * log(|det(mat_b)| + 1e-3)
  mat_b = onehot(y)^T @ softmax(pred[:, :3]) / j_b   (over the valid prefix)

Sharding: pure data parallel, 32 samples per core.  Inputs are host-packed
to fp16 (labels {0,1,2,3} exact), halving HBM traffic; per-(sample,quarter)
partial reductions land in f32 accumulator columns which the host combines
(3x3 dets in f64).  Validated in fp64-vs-fp16 simulation: all 256 det signs
preserved with >10x margin, total rel err ~3e-6.

Layout on core: partition p = b_local*4 + hi (hi = token-axis quarter), free
dim = token-in-chunk.  The key trick is the *min-ramp* reduction: with
h_d = softmax3_d + y packed in one fp16 plane, the per-class masked sums
  m_cd = sum_{y=c} q_d
fall out of differences of ramp sums  Rc = sum min(h_d, c),  c = 1,2,3 --
and each ramp is a single tensor_scalar(op0=min, reduce-op1=add, accum_out)
instruction which qualifies for the DVE 4x_2p fast mode (two-tensor DVE ops
only get 2x at best, and scalar_tensor_tensor gets no fast mode at all).
tensor_scalar reduce semantics (probed on HW): out = in0 op0 s1;
accum_out = s2 + reduce_op1(out).

Per chunk:
  ACT:  ln3_{k-1}=ln(s3), rec_{k-1}=exp(-ln3) [reciprocal via tables],
        e_k = exp(pred_k) (one fused [128,4F] pass), l4_{k-1}=ln(s4),
        3x copy-with-accum_out of the picked-logit products (pk_c)
  DVE:  eq_c = (y==c) with n_c accum riders [ts 4x], tk_c = eq_c*pred_c
        [tensor_tensor 2x], class sums s01/s3/s4 [TT], q_d = e_d*rec [TT],
        h_d = q_d + y [TT], 6 min-ramps [ts 4x], vl = (y<3)*ln(s4) [stt]
Cross-engine dependencies are software-pipelined one chunk deep so neither
in-order engine stalls on the other; DVE and ACT write disjoint accumulator
tiles (no cross-engine WAW).
"""

import numpy as np

import concourse.bass as bass
import concourse.bacc as bacc
import concourse.tile as tile
from concourse import mybir
from concourse.bass_utils import run_bass_kernel_spmd

N_CORES = 8
B, C, L = 256, 4, 16384
B_LOC = B // N_CORES  # 32 samples per core
HI = 4                # token-axis quarters per sample; partition p = b*HI + hi
M = L // HI           # 4096 tokens per partition row
CHUNKS = [1024, 1536, 1536]
NCHUNK = len(CHUNKS)

# accD columns per chunk (DVE): n0,n1,n2, R1_0,R2_0,R3_0, R1_1,R2_1,R3_1, vl
DW = 10
# accA columns per chunk (ACT): pk0, pk1, pk2
AW = 3

IGNORE = 3
DMICE_P = 0.1

# test.py toggles TRACE to get exec_time_ns out of the NTFF profile.
TRACE = False
LAST_EXEC_NS = None
LAST_TRACE_PATH = None

_CACHE = {}

ACT_SET = "natural_log_exp_and_others"  # holds Exp, Ln and Copy together


class _Bacc(bacc.Bacc):
    """Bacc whose act-table pass sees only one (correctly-indexed) set.

    The stock pass resolves each activation to the first table set
    containing its function, which ping-pongs Exp<->Ln between different
    sets and inserts a ~1.3us ACT_TABLE_LOAD per transition.  All functions
    this kernel uses live together in natural_log_exp_and_others, so
    present every other set as empty; ids stay positional, so the emitted
    act_func_set_id is unchanged.
    """

    def insert_act_table_loads(self):
        from concourse import mybir as _mb
        from concourse.hw_specs import get_activation_tables
        import bass_rust as _bass_rust
        has_activation = any(
            isinstance(i, _mb.InstActivation)
            for b in self.main_func.blocks
            for i in b.instructions
        )
        if not has_activation:
            return
        tables = [
            (name, funcs if name == ACT_SET else set())
            for name, funcs in get_activation_tables(self.m.arch).items()
        ]
        _bass_rust.insert_act_table_loads(self, tables)


def _build():
    f32 = mybir.dt.float32
    f16 = mybir.dt.float16
    Alu = mybir.AluOpType
    Act = mybir.ActivationFunctionType

    nc = _Bacc("TRN2", debug=False, target_bir_lowering=False,
               num_devices=N_CORES)
    pred_d = nc.dram_tensor("pred", [B_LOC, C, L], f16, kind="ExternalInput")
    lab_d = nc.dram_tensor("labels", [B_LOC, L], f16, kind="ExternalInput")
    accd_d = nc.dram_tensor("accD", [128, NCHUNK * DW], f32,
                            kind="ExternalOutput")
    acca_d = nc.dram_tensor("accA", [128, NCHUNK * AW], f32,
                            kind="ExternalOutput")

    pred_v = pred_d.ap().rearrange("b c (h m) -> b h c m", h=HI)
    lab_v = lab_d.ap().rearrange("b (h m) -> b h m", h=HI)

    with tile.TileContext(nc) as tc:
        with (
            tc.tile_pool(name="io", bufs=2) as io_pool,
            tc.tile_pool(name="ep", bufs=2) as e_pool,
            tc.tile_pool(name="mid2", bufs=2) as mid2_pool,
            tc.tile_pool(name="mid1", bufs=1) as mid1_pool,
            tc.tile_pool(name="scr", bufs=1) as scr_pool,
            tc.tile_pool(name="accp", bufs=1) as acc_pool,
        ):
            accD = acc_pool.tile([128, NCHUNK * DW], f32)
            accA = acc_pool.tile([128, NCHUNK * AW], f32)
            FMAX = max(CHUNKS)
            # DVE-only scratch for elementwise outputs of reduction ops
            # (same-engine WAW is free on the in-order queue)
            scrD = scr_pool.tile([128, FMAX], f16)

            st = {}  # per-chunk tiles

            def emit_act_pre(j):
                # ln3 and rec for chunk j (before this iter's big exp)
                s = st[j]
                nc.scalar.activation(s["ln3"][:], s["s3"][:], Act.Ln)
                nc.scalar.activation(s["rec"][:], s["ln3"][:], Act.Exp,
                                     scale=-1.0)

            def emit_act_post(j):
                # l4 then the three pk copy-accumulates for chunk j
                s = st[j]
                nc.scalar.activation(s["l4"][:], s["s4"][:], Act.Ln)
                for c in range(3):
                    nc.scalar.activation(
                        s["scrA"][:], s["tk"][c][:], Act.Copy,
                        accum_out=accA[:, j * AW + c:j * AW + c + 1])

            def emit_dve_lag(j):
                # q, h, min-ramps, vl for chunk j (needs rec_j / l4_j)
                s = st[j]
                F = CHUNKS[j]
                nc.vector.tensor_tensor(
                    s["q0"][:], s["et"][:, 0, :], s["rec"][:], Alu.mult)
                nc.vector.tensor_tensor(
                    s["q1"][:], s["et"][:, 1, :], s["rec"][:], Alu.mult)
                nc.vector.tensor_tensor(
                    s["h0"][:], s["q0"][:], s["yt"][:], Alu.add)
                nc.vector.tensor_tensor(
                    s["h1"][:], s["q1"][:], s["yt"][:], Alu.add)
                for d, h in ((0, s["h0"]), (1, s["h1"])):
                    for ci, cap in enumerate((1.0, 2.0, 3.0)):
                        col = j * DW + 3 + 3 * d + ci
                        nc.vector.tensor_scalar(
                            scrD[:, :F], h[:], cap, 0.0, Alu.min, Alu.add,
                            accum_out=accD[:, col:col + 1])
                nc.vector.scalar_tensor_tensor(
                    scrD[:, :F], s["yt"][:], float(IGNORE), s["l4"][:],
                    Alu.is_lt, Alu.mult,
                    accum_out=accD[:, j * DW + 9:j * DW + 10])

            lo = 0
            for k, F in enumerate(CHUNKS):
                s = st[k] = {}
                yt = io_pool.tile([128, F], f16, tag="yt", name="yt")
                pt = io_pool.tile([128, C, F], f16, tag="pt", name="pt")
                s["yt"], s["pt"] = yt, pt
                nc.sync.dma_start(out=yt[:], in_=lab_v[:, :, lo:lo + F])
                for c in range(C):
                    nc.sync.dma_start(out=pt[:, c, :],
                                      in_=pred_v[:, :, c, lo:lo + F])
                lo += F

                # --- ACT: lagged ln3/rec, then this chunk's fused exp
                if k >= 1:
                    emit_act_pre(k - 1)
                et = e_pool.tile([128, C, F], f16, tag="et", name="et")
                s["et"] = et
                nc.scalar.activation(et[:, :, :], pt[:, :, :], Act.Exp)
                if k >= 1:
                    emit_act_post(k - 1)

                # --- DVE: DMA-dependent work first (runs during exp)
                s["eq"] = []
                s["tk"] = []
                for c in range(3):
                    eq = mid1_pool.tile([128, F], f16, tag=f"eq{c}",
                                        name="eq")
                    s["eq"].append(eq)
                    nc.vector.tensor_scalar(
                        eq[:], yt[:], float(c), 0.0, Alu.is_equal, Alu.add,
                        accum_out=accD[:, k * DW + c:k * DW + c + 1])
                for c in range(3):
                    tk = mid2_pool.tile([128, F], f16, tag=f"tk{c}",
                                        name="tk")
                    s["tk"].append(tk)
                    nc.vector.tensor_tensor(
                        tk[:], s["eq"][c][:], pt[:, c, :], Alu.mult)

                # --- DVE: lagged q/h/ramps/vl for previous chunk
                if k >= 1:
                    emit_dve_lag(k - 1)

                # --- DVE: class sums for this chunk (after exp)
                s01 = mid1_pool.tile([128, F], f16, tag="s01", name="s01")
                s3 = mid2_pool.tile([128, F], f16, tag="s3", name="s3")
                s4 = mid2_pool.tile([128, F], f16, tag="s4", name="s4")
                s["s3"], s["s4"] = s3, s4
                nc.vector.tensor_tensor(s01[:], et[:, 0, :], et[:, 1, :],
                                        Alu.add)
                nc.vector.tensor_tensor(s3[:], s01[:], et[:, 2, :], Alu.add)
                nc.vector.tensor_tensor(s4[:], s3[:], et[:, 3, :], Alu.add)

                for nm in ("ln3", "rec", "l4"):
                    s[nm] = mid2_pool.tile([128, F], f16, tag=nm, name=nm)
                for nm in ("q0", "q1", "h0", "h1"):
                    s[nm] = mid1_pool.tile([128, F], f16, tag=nm, name=nm)
                s["scrA"] = mid1_pool.tile([128, F], f16, tag="scrA",
                                           name="scrA")

            last = NCHUNK - 1
            emit_act_pre(last)
            emit_act_post(last)
            emit_dve_lag(last)

            nc.sync.dma_start(out=accd_d.ap(), in_=accD[:])
            nc.sync.dma_start(out=acca_d.ap(), in_=accA[:])
    nc.compile()
    return nc


def _get_nc():
    if "nc" not in _CACHE:
        _CACHE["nc"] = _build()
    return _CACHE["nc"]


def _finalize(accd_list, acca_list):
    """Per-core [128, 3*10] + [128, 3*3] f32 -> scalar loss (f64 host)."""
    per_d, per_a = [], []
    for a in accd_list:
        per_d.append(a.astype(np.float64)
                     .reshape(B_LOC, HI, NCHUNK, DW).sum(axis=(1, 2)))
    for a in acca_list:
        per_a.append(a.astype(np.float64)
                     .reshape(B_LOC, HI, NCHUNK, AW).sum(axis=(1, 2)))
    ad = np.concatenate(per_d, axis=0)   # [256, 10]
    aa = np.concatenate(per_a, axis=0)   # [256, 3]

    n = ad[:, 0:3]                       # per-class valid-token counts
    vl_total = ad[:, 9].sum()
    pk_total = aa.sum()
    j = n.sum(axis=1)
    n3 = float(L) - j                    # pad counts per sample

    # Unpack min-ramp sums: R_c = sum min(h, c) over all tokens, K=1:
    #   R_c = sum_{c'<c} (m_{c'd} + c' n_{c'}) + c * N_{>=c}
    Nge = [j + n3, n[:, 1] + n[:, 2] + n3, n[:, 2] + n3, n3]
    mat = np.zeros((B, 3, 3))
    for d in range(2):
        R = [np.zeros(B)] + [ad[:, 3 + 3 * d + ci] for ci in range(3)]
        for c in range(3):
            mat[:, c, d] = (R[c + 1] - R[c] - c * n[:, c]
                            - ((c + 1) * Nge[c + 1] - c * Nge[c]))
    mat[:, :, 2] = n - mat[:, :, 0] - mat[:, :, 1]
    mat /= j[:, None, None]
    det = np.linalg.det(mat)
    dmi = np.where(det < 0, np.log(np.abs(det) + 1e-3),
                   -np.log(np.abs(det) + 1e-3))
    ce = (vl_total - pk_total) / j.sum()
    loss = DMICE_P * (dmi.sum() / B) + ce
    return np.asarray(loss, dtype=np.float32)


def kernel(pred, labels):
    global LAST_EXEC_NS, LAST_TRACE_PATH
    pred = np.asarray(pred, dtype=np.float32).astype(np.float16)
    labels = np.asarray(labels, dtype=np.int32).astype(np.float16)
    assert pred.shape == (B, C, L) and labels.shape == (B, L)
    nc = _get_nc()
    in_maps = [
        {
            "pred": np.ascontiguousarray(pred[i * B_LOC:(i + 1) * B_LOC]),
            "labels": np.ascontiguousarray(labels[i * B_LOC:(i + 1) * B_LOC]),
        }
        for i in range(N_CORES)
    ]
    res = run_bass_kernel_spmd(nc, in_maps, core_ids=list(range(N_CORES)),
                               trace=TRACE)
    LAST_EXEC_NS = res.exec_time_ns
    if res.instructions_and_trace is not None:
        LAST_TRACE_PATH = res.instructions_and_trace[1]
    return _finalize([r["accD"] for r in res.results],
                     [r["accA"] for r in res.results])


if __name__ == "__main__":
    nc = _build()
    print("build ok")
